# revision 8
# baseline (speedup 1.0000x reference)
"""Trainium2 Bass kernel for the DAM train-batch loss (scatter_memory problem).

The graded end-to-end time is dominated by host->device input staging, so
inputs are aggressively minimized:

  - A_logits ships as fp8e4m3, pre-transposed (contraction axis i on
    partitions) and pre-masked on the host with -240 outside the causal
    triangle (exp(-240) == 0): no device transposes or mask multiplies.
  - Positions are interleaved across cores (core k, slot m handles
    n = 64m + 8k + 1 + r, r in 0..7) so every core's causal-prefix
    footprint is identical: slot m only needs columns i < 64(m+1).  This
    ships just the (padded) triangle: 1.18 MB/core instead of 2.1 MB.
  - phi = softmax(B_logits) @ memory.T is computed on the host (tiny GEMM)
    and shipped as bf16 instead of shipping the 1 MB memory tensor.
  - All matmuls run in bf16; fp8 A adds no measurable error (softmax
    normalization absorbs the quantization noise).

Per core, for each of its 64 positions n:

  EA       = exp(masked A_logits[n].T)              (i, h)  fp8 -> bf16
  nt       = EA.T @ [seq.T | ones]                  (H, 257)  [matmul]
  hatT     = nt[:, :256] / nt[:, 256]               (H, B)    [DVE scale]
  score    = hatT.T @ phi                           (B, M)    [matmul]
  den/num  = sum_m exp(score) {*, plus[m,n]}        (B,)      [ACT + DVE]
  bce sum  = sum_b log(0.5 + targ*(num/den - 0.5))  partial per b

The final mean is assembled on the host from tiny per-core partials.
"""

import sys

sys.path.insert(0, "/opt/trn_rl_repo")

from contextlib import ExitStack

import ml_dtypes
import numpy as np

import concourse.bacc as bacc
import concourse.bass as bass
import concourse.tile as tile
from concourse import mybir
from concourse.bass_utils import run_bass_kernel_spmd

F32 = mybir.dt.float32
BF16 = mybir.dt.bfloat16
FP8 = mybir.dt.float8e4
BF = ml_dtypes.bfloat16
F8 = ml_dtypes.float8_e4m3

N = 512          # sequence length
H = 64           # heads
M = 1024         # memories
B = 256          # batch
NL = 64          # positions per core
NS = 8           # slots per core (8 positions each)
NCORES = 8
MASKVAL = -240.0  # fp8e4m3-representable; exp() flushes to 0

Exp = mybir.ActivationFunctionType.Exp
Ln = mybir.ActivationFunctionType.Ln
MULT = mybir.AluOpType.mult
SUB = mybir.AluOpType.subtract

_NC = None


def _slot_chunks(m):
    """i-chunks covering [0, 64*(m+1)): full-128 chunks plus one 64 tail."""
    C = 64 * (m + 1)
    out = []
    i0 = 0
    while i0 < C:
        pc = min(128, C - i0)
        out.append((i0 // 128, pc))
        i0 += pc
    return out


def _build():
    global _NC
    if _NC is not None:
        return _NC

    nc = bacc.Bacc("TRN2", target_bir_lowering=False)

    # per-slot triangle blocks: a{m} is [64*(m+1), 8*64] fp8, rows = i
    a_d = [
        nc.dram_tensor(f"a{m}", [64 * (m + 1), 8 * H], FP8, kind="ExternalInput")
        for m in range(NS)
    ]
    # [p, (ic, b|1|0)] bf16 shard: rows 16k..16k+16 of the full [128, 4*258]
    sqT = nc.dram_tensor("sqT", [16, 4 * 258], BF16, kind="ExternalInput")
    phi = nc.dram_tensor("phi", [H // 8, M], BF16, kind="ExternalInput")
    plus = nc.dram_tensor("plus", [NL, M], FP8, kind="ExternalInput")
    # packed bf16 tail tensors: [p, (tg_c0, tg_c1, cw)]
    tgw = nc.dram_tensor("tgw", [128, 3 * NL], BF16, kind="ExternalInput")
    part_out = nc.dram_tensor("partial", [2, 128], F32, kind="ExternalOutput")

    with tile.TileContext(nc) as tc, ExitStack() as ctx:
        consts = ctx.enter_context(tc.tile_pool(name="consts", bufs=1))
        accs = ctx.enter_context(tc.tile_pool(name="accs", bufs=1))
        abuf = ctx.enter_context(tc.tile_pool(name="abuf", bufs=2))
        eabuf = ctx.enter_context(tc.tile_pool(name="eabuf", bufs=2))
        hatb = ctx.enter_context(tc.tile_pool(name="hatb", bufs=3))
        ebuf = ctx.enter_context(tc.tile_pool(name="ebuf", bufs=3))
        pbuf = ctx.enter_context(tc.tile_pool(name="pbuf", bufs=2))
        scr = ctx.enter_context(tc.tile_pool(name="scr", bufs=3))
        ntpsum = ctx.enter_context(tc.tile_pool(name="ntpsum", bufs=2, space="PSUM"))
        scpsum = ctx.enter_context(tc.tile_pool(name="scpsum", bufs=2, space="PSUM"))

        dram = ctx.enter_context(tc.tile_pool(name="dram", bufs=1, space="DRAM"))

        # ---- all-gather the replicated constants from 1/8 shards ----
        sq_in = dram.tile([16, 4 * 258], BF16)
        sq_g = dram.tile([128, 4 * 258], BF16)
        nc.gpsimd.dma_start(sq_in[:], sqT[:])
        nc.gpsimd.collective_compute(
            "AllGather", mybir.AluOpType.bypass,
            replica_groups=[list(range(NCORES))],
            ins=[sq_in.opt()], outs=[sq_g.opt()],
        )
        phi_in = dram.tile([H // 8, M], BF16)
        phi_g = dram.tile([H, M], BF16)
        nc.gpsimd.dma_start(phi_in[:], phi[:])
        nc.gpsimd.collective_compute(
            "AllGather", mybir.AluOpType.bypass,
            replica_groups=[list(range(NCORES))],
            ins=[phi_in.opt()], outs=[phi_g.opt()],
        )

        # ---- constants ----
        sq_sb = consts.tile([128, 4, 258], BF16)
        nc.sync.dma_start(sq_sb[:], sq_g[:])
        phi_sb = consts.tile([H, M], BF16)
        nc.sync.dma_start(phi_sb[:], phi_g[:])
        tgw_sb = consts.tile([128, 3 * NL], BF16)
        nc.sync.dma_start(tgw_sb[:], tgw[:])

        den_sb = accs.tile([128, 2, NL], F32)
        num_sb = accs.tile([128, 2, NL], F32)

        # ---- main loop: slots of 8 positions ----
        for m in range(NS):
            chunks = _slot_chunks(m)
            nch = len(chunks)
            # load + exp this slot's triangle block
            EA = eabuf.tile([128, 4, 8 * H], BF16, tag="EA")
            for ci, (ic, pc) in enumerate(chunks):
                at = abuf.tile([128, 8 * H], FP8, tag="at")
                nc.sync.dma_start(
                    at[0:pc, :], a_d[m][ic * 128:ic * 128 + pc, :]
                )
                nc.scalar.activation(EA[0:pc, ci, :], at[0:pc, :], Exp)

            for r in range(8):
                j = 8 * m + r
                nt_ps = ntpsum.tile([H, 258], F32, tag="nt")
                for ci, (ic, pc) in enumerate(chunks):
                    nc.tensor.matmul(
                        nt_ps[:],
                        lhsT=EA[0:pc, ci, r * H:(r + 1) * H],
                        rhs=sq_sb[0:pc, ic, :],
                        start=(ci == 0),
                        stop=(ci == nch - 1),
                    )
                dinv = hatb.tile([H, 1], F32, tag="dinv")
                nc.vector.reciprocal(dinv[:], nt_ps[:, 256:257])
                hatT = hatb.tile([H, B], BF16, tag="hat")
                nc.vector.tensor_scalar_mul(hatT[:], nt_ps[:, 0:B], dinv[:])

                # broadcast plus row j across 128 partitions (DRAM-sourced)
                pb = pbuf.tile([128, M], BF16)
                row = plus[j:j + 1, :]
                src = bass.AP(
                    tensor=row.tensor, offset=row.offset,
                    ap=[[0, 128]] + [list(d) for d in row.ap[1:]],
                )
                nc.gpsimd.dma_start(pb[:], src)

                for c in range(2):
                    sc_ps = scpsum.tile([128, M], F32, tag="scps")
                    for mh in range(2):
                        nc.tensor.matmul(
                            sc_ps[:, mh * 512:(mh + 1) * 512],
                            lhsT=hatT[:, c * 128:(c + 1) * 128],
                            rhs=phi_sb[:, mh * 512:(mh + 1) * 512],
                            start=True,
                            stop=True,
                        )
                    E_t = ebuf.tile([128, M], BF16)
                    nc.scalar.activation(
                        E_t[:], sc_ps[:], Exp,
                        accum_out=den_sb[:, c, j:j + 1],
                    )
                    sout = scr.tile([128, M], BF16)
                    nc.vector.scalar_tensor_tensor(
                        out=sout[:],
                        in0=E_t[:],
                        scalar=1.0,
                        in1=pb[:],
                        op0=MULT,
                        op1=MULT,
                        accum_out=num_sb[:, c, j:j + 1],
                    )

        # ---- tail: bce partials ----
        half_sb = accs.tile([128, 1], F32)
        nc.vector.memset(half_sb[:], 0.5)
        tgw_f = accs.tile([128, 3 * NL], F32)
        nc.vector.tensor_copy(tgw_f[:], tgw_sb[:])
        for c in range(2):
            rec = accs.tile([128, NL], F32, tag=f"rec{c}")
            nc.vector.reciprocal(rec[:], den_sb[:, c, :])
            pr = accs.tile([128, NL], F32, tag=f"pr{c}")
            nc.vector.tensor_mul(pr[:], num_sb[:, c, :], rec[:])
            nc.vector.tensor_scalar_max(pr[:], pr[:], 1e-6)
            nc.vector.tensor_scalar_min(pr[:], pr[:], 1.0 - 1e-6)
            qq = accs.tile([128, NL], F32, tag=f"qq{c}")
            nc.vector.scalar_tensor_tensor(
                out=qq[:], in0=pr[:], scalar=0.5, in1=tgw_f[:, c * NL:(c + 1) * NL],
                op0=SUB, op1=MULT,
            )
            lg = accs.tile([128, NL], F32, tag=f"lg{c}")
            nc.scalar.activation(lg[:], qq[:], Ln, bias=half_sb[:])
            ws = accs.tile([128, NL], F32, tag=f"ws{c}")
            rs = accs.tile([128, 1], F32, tag=f"rs{c}")
            nc.vector.scalar_tensor_tensor(
                out=ws[:], in0=lg[:], scalar=1.0, in1=tgw_f[:, 2 * NL:3 * NL],
                op0=MULT, op1=MULT, accum_out=rs[:],
            )
            nc.sync.dma_start(part_out[c:c + 1, :], rs[:, 0:1])

    nc.compile()
    _NC = nc
    return nc


def _core_positions(k):
    """Global n for core k, local order j = 8m + r."""
    return np.array([64 * m + 8 * k + 1 + r for m in range(NS) for r in range(8)])


def _in_maps(sequences, memory, A_logits, B_logits):
    sequences = np.asarray(sequences, np.float32)
    memory = np.asarray(memory, np.float32)
    A_logits = np.asarray(A_logits, np.float32)
    B_logits = np.asarray(B_logits, np.float32)

    # host phi = softmax(B_logits) @ memory.T  (tiny)
    eb = np.exp(B_logits - B_logits.max(-1, keepdims=True))
    Bn = eb / eb.sum(-1, keepdims=True)
    phi_full = (Bn @ memory.T).astype(BF)                     # (64, 1024)
    phi_shards = [np.ascontiguousarray(phi_full[8 * k:8 * (k + 1)]) for k in range(NCORES)]

    sqT_full = np.concatenate(
        [sequences.T, np.ones((N, 1), np.float32), np.zeros((N, 1), np.float32)],
        axis=1,
    ).astype(BF)                                              # (512, 258)
    sq_pack = np.ascontiguousarray(
        sqT_full.reshape(4, 128, 258).transpose(1, 0, 2)
    ).reshape(128, 4 * 258)
    sq_shards = [np.ascontiguousarray(sq_pack[16 * k:16 * (k + 1)]) for k in range(NCORES)]

    A8 = A_logits.astype(F8)                                  # fp8 once, globally
    mask_fill = F8(MASKVAL)

    maps = []
    for k in range(NCORES):
        n_real = _core_positions(k)              # (64,), may include 512
        ns = np.minimum(n_real, N - 1)
        mp = {}
        for m in range(NS):
            C = 64 * (m + 1)
            nr = n_real[8 * m:8 * m + 8]
            nss = ns[8 * m:8 * m + 8]
            blk = A8[nss][:, :, :C]              # (8, 64, C) fp8
            blkT = np.ascontiguousarray(blk.transpose(2, 0, 1))  # (C, 8, 64)
            m_bad = np.arange(C)[:, None] >= nr[None, :]         # (C, 8)
            blkT[m_bad] = mask_fill
            mp[f"a{m}"] = blkT.reshape(C, 8 * H)

        pl = np.ascontiguousarray((memory[:, ns].T > 0)).astype(F8)  # (64, 1024)
        t_raw = sequences[:, ns].copy()          # (256, 64)
        w = np.ones((128, NL), np.float32)
        pad = n_real > (N - 1)
        t_raw[:, pad] = 0.0
        w[:, pad] = 0.0
        tgw = np.empty((128, 3, NL), np.float32)
        tgw[:, 0, :] = t_raw[0:128]
        tgw[:, 1, :] = t_raw[128:256]
        tgw[:, 2, :] = w
        mp.update({
            "sqT": sq_shards[k],
            "phi": phi_shards[k],
            "plus": pl,
            "tgw": tgw.reshape(128, 3 * NL).astype(BF),
        })
        maps.append(mp)
    return maps


def _run(maps, trace=False):
    nc = _build()
    return run_bass_kernel_spmd(nc, maps, list(range(NCORES)), trace=trace)


def kernel(sequences, memory, A_logits, B_logits, _trace=False):
    maps = _in_maps(sequences, memory, A_logits, B_logits)
    res = _run(maps, trace=_trace)
    tot = 0.0
    for r in res.results:
        tot += r["partial"].astype(np.float64).sum()
    out = np.float32(-tot / (B * (N - 1)))
    if _trace:
        return out, res
    return out


# revision 10
# speedup vs baseline: 1.0667x; 1.0667x over previous
"""Trainium2 Bass kernel for the DAM train-batch loss (scatter_memory problem).

The graded end-to-end time is dominated by host->device input staging, so
inputs are aggressively minimized:

  - A_logits ships as fp8e4m3, pre-transposed (contraction axis i on
    partitions) and pre-masked on the host with -240 outside the causal
    triangle (exp(-240) == 0): no device transposes or mask multiplies.
  - Positions are interleaved across cores (core k, slot m handles
    n = 64m + 8k + 1 + r, r in 0..7) so every core's causal-prefix
    footprint is identical: slot m only needs columns i < 64(m+1).  This
    ships just the (padded) triangle: 1.18 MB/core instead of 2.1 MB.
  - phi = softmax(B_logits) @ memory.T is computed on the host (tiny GEMM)
    and shipped as bf16 instead of shipping the 1 MB memory tensor.
  - All matmuls run in bf16; fp8 A adds no measurable error (softmax
    normalization absorbs the quantization noise).

Per core, for each of its 64 positions n:

  EA       = exp(masked A_logits[n].T)              (i, h)  fp8 -> bf16
  nt       = EA.T @ [seq.T | ones]                  (H, 257)  [matmul]
  hatT     = nt[:, :256] / nt[:, 256]               (H, B)    [DVE scale]
  score    = hatT.T @ phi                           (B, M)    [matmul]
  den/num  = sum_m exp(score) {*, plus[m,n]}        (B,)      [ACT + DVE]
  bce sum  = sum_b log(0.5 + targ*(num/den - 0.5))  partial per b

The final mean is assembled on the host from tiny per-core partials.
"""

import sys

sys.path.insert(0, "/opt/trn_rl_repo")

from contextlib import ExitStack

import ml_dtypes
import numpy as np

import concourse.bacc as bacc
import concourse.bass as bass
import concourse.tile as tile
from concourse import mybir
from concourse.bass_utils import run_bass_kernel_spmd

F32 = mybir.dt.float32
BF16 = mybir.dt.bfloat16
FP8 = mybir.dt.float8e4
BF = ml_dtypes.bfloat16
F8 = ml_dtypes.float8_e4m3

N = 512          # sequence length
H = 64           # heads
M = 1024         # memories
B = 256          # batch
NL = 64          # positions per core
NS = 8           # slots per core (8 positions each)
NCORES = 8
MASKVAL = -240.0  # fp8e4m3-representable; exp() flushes to 0

Exp = mybir.ActivationFunctionType.Exp
Ln = mybir.ActivationFunctionType.Ln
MULT = mybir.AluOpType.mult
SUB = mybir.AluOpType.subtract

_NC = None


def _slot_chunks(m):
    """i-chunks covering [0, 64*(m+1)): full-128 chunks plus one 64 tail."""
    C = 64 * (m + 1)
    out = []
    i0 = 0
    while i0 < C:
        pc = min(128, C - i0)
        out.append((i0 // 128, pc))
        i0 += pc
    return out


def _build():
    global _NC
    if _NC is not None:
        return _NC

    nc = bacc.Bacc("TRN2", target_bir_lowering=False)

    # per-slot triangle blocks: a{m} is [64*(m+1), 8*64] fp8, rows = i
    a_d = [
        nc.dram_tensor(f"a{m}", [64 * (m + 1), 8 * H], FP8, kind="ExternalInput")
        for m in range(NS)
    ]
    # [p, (ic, b|1|0)] bf16 shard: rows 16k..16k+16 of the full [128, 4*258]
    sqT = nc.dram_tensor("sqT", [16, 4 * 258], BF16, kind="ExternalInput")
    phi = nc.dram_tensor("phi", [H // 8, M], BF16, kind="ExternalInput")
    plus = nc.dram_tensor("plus", [NL, M], FP8, kind="ExternalInput")
    # packed bf16 tail tensors: [p, (tg_c0, tg_c1, cw)]
    tgw = nc.dram_tensor("tgw", [128, 3 * NL], BF16, kind="ExternalInput")
    part_out = nc.dram_tensor("partial", [2, 128], F32, kind="ExternalOutput")

    with tile.TileContext(nc) as tc, ExitStack() as ctx:
        consts = ctx.enter_context(tc.tile_pool(name="consts", bufs=1))
        accs = ctx.enter_context(tc.tile_pool(name="accs", bufs=1))
        abuf = ctx.enter_context(tc.tile_pool(name="abuf", bufs=2))
        eabuf = ctx.enter_context(tc.tile_pool(name="eabuf", bufs=2))
        hatb = ctx.enter_context(tc.tile_pool(name="hatb", bufs=3))
        ebuf = ctx.enter_context(tc.tile_pool(name="ebuf", bufs=3))
        pbuf = ctx.enter_context(tc.tile_pool(name="pbuf", bufs=2))
        scr = ctx.enter_context(tc.tile_pool(name="scr", bufs=3))
        ntpsum = ctx.enter_context(tc.tile_pool(name="ntpsum", bufs=2, space="PSUM"))
        scpsum = ctx.enter_context(tc.tile_pool(name="scpsum", bufs=1, space="PSUM"))

        dram = ctx.enter_context(tc.tile_pool(name="dram", bufs=1, space="DRAM"))

        # ---- all-gather the replicated constants from 1/8 shards ----
        sq_in = dram.tile([16, 4 * 258], BF16)
        sq_g = dram.tile([128, 4 * 258], BF16)
        nc.gpsimd.dma_start(sq_in[:], sqT[:])
        nc.gpsimd.collective_compute(
            "AllGather", mybir.AluOpType.bypass,
            replica_groups=[list(range(NCORES))],
            ins=[sq_in.opt()], outs=[sq_g.opt()],
        )
        phi_in = dram.tile([H // 8, M], BF16)
        phi_g = dram.tile([H, M], BF16)
        nc.gpsimd.dma_start(phi_in[:], phi[:])
        nc.gpsimd.collective_compute(
            "AllGather", mybir.AluOpType.bypass,
            replica_groups=[list(range(NCORES))],
            ins=[phi_in.opt()], outs=[phi_g.opt()],
        )

        # ---- constants ----
        sq_sb = consts.tile([128, 4, 258], BF16)
        nc.sync.dma_start(sq_sb[:], sq_g[:])
        phi_sb = consts.tile([128, M], BF16)
        nc.sync.dma_start(phi_sb[0:H, :], phi_g[:])
        nc.sync.dma_start(phi_sb[H:128, :], phi_g[:])
        tgw_sb = consts.tile([128, 3 * NL], BF16)
        nc.sync.dma_start(tgw_sb[:], tgw[:])

        den_sb = accs.tile([128, 2, NL], F32)
        num_sb = accs.tile([128, 2, NL], F32)

        # ---- main loop: slots of 8 positions ----
        for m in range(NS):
            chunks = _slot_chunks(m)
            nch = len(chunks)
            # load + exp this slot's triangle block
            EA = eabuf.tile([128, 4, 8 * H], BF16, tag="EA")
            for ci, (ic, pc) in enumerate(chunks):
                at = abuf.tile([128, 8 * H], FP8, tag="at")
                nc.sync.dma_start(
                    at[0:pc, :], a_d[m][ic * 128:ic * 128 + pc, :]
                )
                nc.scalar.activation(EA[0:pc, ci, :], at[0:pc, :], Exp)

            for rr in range(4):
                # two positions share one [128, 258] nt tile: rows
                # 0-63 = position 2rr, 64-127 = position 2rr+1
                nt_ps = ntpsum.tile([128, 258], F32, tag="nt")
                for ci, (ic, pc) in enumerate(chunks):
                    nc.tensor.matmul(
                        nt_ps[:],
                        lhsT=EA[0:pc, ci, 2 * rr * H:(2 * rr + 2) * H],
                        rhs=sq_sb[0:pc, ic, :],
                        start=(ci == 0),
                        stop=(ci == nch - 1),
                    )
                dinv = hatb.tile([128, 1], F32, tag="dinv")
                nc.vector.reciprocal(dinv[:], nt_ps[:, 256:257])
                hatT = hatb.tile([128, B], BF16, tag="hat")
                nc.vector.tensor_scalar_mul(hatT[:], nt_ps[:, 0:B], dinv[:])

                pbs = []
                for pos in range(2):
                    j = 8 * m + 2 * rr + pos
                    pb = pbuf.tile([128, M], BF16, tag=f"pb{pos}")
                    row = plus[j:j + 1, :]
                    src = bass.AP(
                        tensor=row.tensor, offset=row.offset,
                        ap=[[0, 128]] + [list(d) for d in row.ap[1:]],
                    )
                    nc.gpsimd.dma_start(pb[:], src)
                    pbs.append(pb)

                for c in range(2):
                    # the two positions' score matmuls use disjoint PE
                    # row-quadrants (lhsT/rhs partition base 0 vs 64) and
                    # run concurrently in the array
                    sc_list = []
                    for pos in range(2):
                        sc_ps = scpsum.tile([128, M], F32, tag=f"scps{pos}")
                        for mh in range(2):
                            nc.tensor.matmul(
                                sc_ps[:, mh * 512:(mh + 1) * 512],
                                lhsT=hatT[pos * H:(pos + 1) * H,
                                          c * 128:(c + 1) * 128],
                                rhs=phi_sb[pos * H:(pos + 1) * H,
                                           mh * 512:(mh + 1) * 512],
                                start=True,
                                stop=True,
                            )
                        sc_list.append(sc_ps)
                    for pos in range(2):
                        j = 8 * m + 2 * rr + pos
                        E_t = ebuf.tile([128, M], BF16)
                        nc.scalar.activation(
                            E_t[:], sc_list[pos][:], Exp,
                            accum_out=den_sb[:, c, j:j + 1],
                        )
                        sout = scr.tile([128, M], BF16)
                        nc.vector.scalar_tensor_tensor(
                            out=sout[:],
                            in0=E_t[:],
                            scalar=1.0,
                            in1=pbs[pos][:],
                            op0=MULT,
                            op1=MULT,
                            accum_out=num_sb[:, c, j:j + 1],
                        )

        # ---- tail: bce partials ----
        half_sb = accs.tile([128, 1], F32)
        nc.vector.memset(half_sb[:], 0.5)
        tgw_f = accs.tile([128, 3 * NL], F32)
        nc.vector.tensor_copy(tgw_f[:], tgw_sb[:])
        for c in range(2):
            rec = accs.tile([128, NL], F32, tag=f"rec{c}")
            nc.vector.reciprocal(rec[:], den_sb[:, c, :])
            pr = accs.tile([128, NL], F32, tag=f"pr{c}")
            nc.vector.tensor_mul(pr[:], num_sb[:, c, :], rec[:])
            nc.vector.tensor_scalar_max(pr[:], pr[:], 1e-6)
            nc.vector.tensor_scalar_min(pr[:], pr[:], 1.0 - 1e-6)
            qq = accs.tile([128, NL], F32, tag=f"qq{c}")
            nc.vector.scalar_tensor_tensor(
                out=qq[:], in0=pr[:], scalar=0.5, in1=tgw_f[:, c * NL:(c + 1) * NL],
                op0=SUB, op1=MULT,
            )
            lg = accs.tile([128, NL], F32, tag=f"lg{c}")
            nc.scalar.activation(lg[:], qq[:], Ln, bias=half_sb[:])
            ws = accs.tile([128, NL], F32, tag=f"ws{c}")
            rs = accs.tile([128, 1], F32, tag=f"rs{c}")
            nc.vector.scalar_tensor_tensor(
                out=ws[:], in0=lg[:], scalar=1.0, in1=tgw_f[:, 2 * NL:3 * NL],
                op0=MULT, op1=MULT, accum_out=rs[:],
            )
            nc.sync.dma_start(part_out[c:c + 1, :], rs[:, 0:1])

    nc.compile()
    _NC = nc
    return nc


def _core_positions(k):
    """Global n for core k, local order j = 8m + r."""
    return np.array([64 * m + 8 * k + 1 + r for m in range(NS) for r in range(8)])


def _in_maps(sequences, memory, A_logits, B_logits):
    sequences = np.asarray(sequences, np.float32)
    memory = np.asarray(memory, np.float32)
    A_logits = np.asarray(A_logits, np.float32)
    B_logits = np.asarray(B_logits, np.float32)

    # host phi = softmax(B_logits) @ memory.T  (tiny)
    eb = np.exp(B_logits - B_logits.max(-1, keepdims=True))
    Bn = eb / eb.sum(-1, keepdims=True)
    phi_full = (Bn @ memory.T).astype(BF)                     # (64, 1024)
    phi_shards = [np.ascontiguousarray(phi_full[8 * k:8 * (k + 1)]) for k in range(NCORES)]

    sqT_full = np.concatenate(
        [sequences.T, np.ones((N, 1), np.float32), np.zeros((N, 1), np.float32)],
        axis=1,
    ).astype(BF)                                              # (512, 258)
    sq_pack = np.ascontiguousarray(
        sqT_full.reshape(4, 128, 258).transpose(1, 0, 2)
    ).reshape(128, 4 * 258)
    sq_shards = [np.ascontiguousarray(sq_pack[16 * k:16 * (k + 1)]) for k in range(NCORES)]

    A8 = A_logits.astype(F8)                                  # fp8 once, globally
    mask_fill = F8(MASKVAL)

    maps = []
    for k in range(NCORES):
        n_real = _core_positions(k)              # (64,), may include 512
        ns = np.minimum(n_real, N - 1)
        mp = {}
        for m in range(NS):
            C = 64 * (m + 1)
            nr = n_real[8 * m:8 * m + 8]
            nss = ns[8 * m:8 * m + 8]
            blk = A8[nss][:, :, :C]              # (8, 64, C) fp8
            blkT = np.ascontiguousarray(blk.transpose(2, 0, 1))  # (C, 8, 64)
            m_bad = np.arange(C)[:, None] >= nr[None, :]         # (C, 8)
            blkT[m_bad] = mask_fill
            mp[f"a{m}"] = blkT.reshape(C, 8 * H)

        pl = np.ascontiguousarray((memory[:, ns].T > 0)).astype(F8)  # (64, 1024)
        t_raw = sequences[:, ns].copy()          # (256, 64)
        w = np.ones((128, NL), np.float32)
        pad = n_real > (N - 1)
        t_raw[:, pad] = 0.0
        w[:, pad] = 0.0
        tgw = np.empty((128, 3, NL), np.float32)
        tgw[:, 0, :] = t_raw[0:128]
        tgw[:, 1, :] = t_raw[128:256]
        tgw[:, 2, :] = w
        mp.update({
            "sqT": sq_shards[k],
            "phi": phi_shards[k],
            "plus": pl,
            "tgw": tgw.reshape(128, 3 * NL).astype(BF),
        })
        maps.append(mp)
    return maps


def _run(maps, trace=False):
    nc = _build()
    return run_bass_kernel_spmd(nc, maps, list(range(NCORES)), trace=trace)


def kernel(sequences, memory, A_logits, B_logits, _trace=False):
    maps = _in_maps(sequences, memory, A_logits, B_logits)
    res = _run(maps, trace=_trace)
    tot = 0.0
    for r in res.results:
        tot += r["partial"].astype(np.float64).sum()
    out = np.float32(-tot / (B * (N - 1)))
    if _trace:
        return out, res
    return out
